# revision 31
# baseline (speedup 1.0000x reference)
"""Trainium2 Bass kernel for a 2-layer Transformer encoder (nn_Encoder).

Sharding: data-parallel over batch (4 pairs of cores) x tensor-parallel
(2-way) within each pair: heads 0-7 / 8-15 for attention, d_ff columns
0-2047 / 2048-4095 for the FFN, with a pairwise AllReduce after the WO
and W2 projections.

Device layout: activations are kept transposed (X_T: [d_model, tokens])
so every matmul contracts over the partition dim. Attention scores are
computed as S_T [keys, queries]; softmax rowsums come for free from a
ones-column appended to V; probabilities are normalized on-device and
written to DRAM in [k, q] layout (host transposes while assembling).

All matmuls run in float32r (fp32 rounded to 11-bit mantissa, full PE
rate); end-to-end max rel err vs the fp32 reference is ~6e-4.
"""
import sys

sys.path.insert(0, "/opt/trn_rl_repo")

import math
import numpy as np

import concourse.bass as bass
import concourse.tile as tile
from concourse import bacc, mybir
from concourse import bass_utils

F32 = mybir.dt.float32
F32R = mybir.dt.float32r

N_CORES = 8
B, L, D, DK, HEADS, DFF, NL = 4, 1024, 1024, 64, 16, 4096, 2
H_PER = HEADS // 2          # heads per core
FH = H_PER * DK             # 512 head-feature rows per core
DFF_PER = DFF // 2          # 2048 ffn cols per core
EPS = 1e-5
NEG = -1e30


def _round_tf(x):
    """Round fp32 -> float32r (RNE to 11-bit mantissa). Matches walrus."""
    x = np.ascontiguousarray(x, dtype=np.float32)
    u = x.view(np.uint32)
    r = (u + np.uint32(0x7FF) + ((u >> np.uint32(12)) & np.uint32(1))) & np.uint32(
        0xFFFFF000
    )
    return r.view(np.float32)


def build(l_seq=L, n_layers=NL, qs=None):
    """Build + compile the per-core SPMD program. Same program on all cores."""
    nq = l_seq                      # queries per core (all of them, TP over heads)
    QS = qs if qs is not None else min(512, nq)  # q-block width
    QB = nq // QS                   # number of query blocks
    KT = l_seq // 128               # 128-row key tiles
    TT = l_seq // 128               # token tiles for V
    DC = D // 128                   # d_model chunks
    FT = FH // 128                  # f-tiles for Q_T/K_T (4)
    HC = FH // 128                  # hdv chunks for WO (4)
    F1 = DFF_PER // 128             # dff tiles (16)
    DM = D // 128                   # output d_model tiles (8)

    nc = bacc.Bacc("TRN2", target_bir_lowering=False, debug=False,
                   num_devices=N_CORES)

    x0 = nc.dram_tensor("x0", [D, l_seq], F32R, kind="ExternalInput").ap()
    mask = nc.dram_tensor("mask", [128, KT], F32, kind="ExternalInput").ap()
    cones = nc.dram_tensor("cones", [128, 128], F32R, kind="ExternalInput").ap()
    wq, wk, wv, wo, w1, w2 = [], [], [], [], [], []
    for l in range(n_layers):
        wq.append(nc.dram_tensor(f"wq{l}", [D, FH], F32R, kind="ExternalInput").ap())
        wk.append(nc.dram_tensor(f"wk{l}", [D, FH], F32R, kind="ExternalInput").ap())
        wv.append(nc.dram_tensor(f"wv{l}", [D, FH], F32R, kind="ExternalInput").ap())
        wo.append(nc.dram_tensor(f"wo{l}", [FH, D], F32R, kind="ExternalInput").ap())
        w1.append(nc.dram_tensor(f"w1{l}", [D, DFF_PER], F32R, kind="ExternalInput").ap())
        w2.append(nc.dram_tensor(f"w2{l}", [DFF_PER, D], F32R, kind="ExternalInput").ap())
    attns = nc.dram_tensor("attns", [n_layers, H_PER, l_seq, nq], F32,
                           kind="ExternalOutput").ap()
    xout = nc.dram_tensor("xout", [D, nq], F32R, kind="ExternalOutput").ap()

    RG = [[0, 1], [2, 3], [4, 5], [6, 7]]

    with tile.TileContext(nc) as tc:
        with (
            tc.tile_pool(name="persist", bufs=1) as pp,
            tc.tile_pool(name="acts", bufs=1) as pa,
            tc.tile_pool(name="wload", bufs=2) as pw,
            tc.tile_pool(name="soft", bufs=1) as psf,
            tc.tile_pool(name="small", bufs=1) as psm,
            tc.tile_pool(name="psum", bufs=2, space="PSUM") as ps,
            tc.tile_pool(name="dram", bufs=4, space="DRAM") as pd,
        ):
            # persistent tiles
            x_sb = [pp.tile([128, l_seq], F32R, tag=f"x{i}", name=f"x{i}")
                    for i in range(DC)]
            mask_sb = pp.tile([128, KT], F32, tag="mask")
            ones_sb = pp.tile([128, 1], F32R, tag="ones")     # LN stats lhsT
            onesr_sb = pp.tile([1, 128], F32R, tag="onesr")   # broadcast lhsT
            onesr64_sb = pp.tile([65, 128], F32R, tag="onesr64")  # row 64 ones
            eps_sb = pp.tile([1, 1], F32, tag="eps")

            nc.sync.dma_start(out=ones_sb, in_=cones[:, 0:1])
            nc.sync.dma_start(out=onesr_sb, in_=cones[0:1, :])
            nc.sync.dma_start(out=onesr64_sb[64:65, :], in_=cones[0:1, :])
            nc.vector.memset(eps_sb, EPS)
            nc.sync.dma_start(out=mask_sb, in_=mask)
            for i in range(DC):
                nc.sync.dma_start(out=x_sb[i], in_=x0[i * 128:(i + 1) * 128, :])

            for l in range(n_layers):
                # ---------------- QKV projections ----------------
                qt_sb = [pa.tile([128, nq], F32R, tag=f"qt{i}", name=f"qt{i}")
                         for i in range(FT)]
                kt_sb = [pa.tile([128, nq], F32R, tag=f"kt{i}", name=f"kt{i}")
                         for i in range(FT)]
                v_sb = [pa.tile([128, H_PER, DK + 1], F32R, tag=f"v{i}",
                                name=f"v{i}") for i in range(TT)]

                for (w_ap, dst) in ((wq[l], qt_sb), (wk[l], kt_sb)):
                    for ft in range(FT):
                        wt = pw.tile([128, DC, 128], F32R, tag="wqk", name="wt")
                        nc.sync.dma_start(
                            out=wt,
                            in_=w_ap[:, ft * 128:(ft + 1) * 128].rearrange(
                                "(c p) n -> p c n", p=128))
                        for qb in range(QB):
                            acc = ps.tile([128, QS], F32, tag="mm", name="acc")
                            for dc in range(DC):
                                nc.tensor.matmul(
                                    acc[:], wt[:, dc, :],
                                    x_sb[dc][:, qb * QS:(qb + 1) * QS],
                                    start=(dc == 0), stop=(dc == DC - 1))
                            nc.vector.tensor_copy(
                                dst[ft][:, qb * QS:(qb + 1) * QS], acc[:])

                # V natural [tok, h*(dk+1)] with ones column per head
                wv_hold = [pa.tile([128, FH], F32R, tag=f"z{dc}", name=f"wvh{dc}")
                           for dc in range(DC)]
                for dc in range(DC):
                    nc.sync.dma_start(out=wv_hold[dc],
                                      in_=wv[l][dc * 128:(dc + 1) * 128, :])
                for tt in range(TT):
                    acc = ps.tile([128, FH], F32, tag="mm", name="acc")
                    for dc in range(DC):
                        nc.tensor.matmul(
                            acc[:], x_sb[dc][:, tt * 128:(tt + 1) * 128],
                            wv_hold[dc][:], start=(dc == 0), stop=(dc == DC - 1))
                    nc.sync.dma_start(out=v_sb[tt][:, :, DK:DK + 1],
                                      in_=cones[:, 0:H_PER].rearrange(
                                          "p (h o) -> p h o", o=1))
                    nc.vector.tensor_copy(
                        v_sb[tt][:, :, 0:DK],
                        acc[:].rearrange("p (h d) -> p h d", h=H_PER))

                # ---------------- attention ----------------
                ctx_sb = [pa.tile([128, nq], F32R, tag=f"ctx{i}", name=f"ctx{i}")
                          for i in range(HC)]
                ar_in = [pd.tile([D, QS], F32, tag=f"arin{qb}", name=f"ari{qb}")
                         for qb in range(QB)]
                ar_out = [pd.tile([D, QS], F32, tag=f"arout{qb}", name=f"aro{qb}")
                          for qb in range(QB)]

                for qb in range(QB):
                    for h in range(H_PER):
                        f, po = h // 2, (h % 2) * 64
                        es = [psf.tile([128, QS], F32R, tag="es", bufs=KT,
                                       name=f"es{kt}") for kt in range(KT)]
                        for kt in range(KT):
                            sps = ps.tile([128, QS], F32, tag="score", name="sps")
                            nc.tensor.matmul(
                                sps[:],
                                kt_sb[f][po:po + 64, kt * 128:(kt + 1) * 128],
                                qt_sb[f][po:po + 64, qb * QS:(qb + 1) * QS],
                                start=True, stop=True)
                            nc.scalar.activation(
                                es[kt][:], sps[:],
                                mybir.ActivationFunctionType.Exp,
                                bias=mask_sb[:, kt:kt + 1], scale=1.0)
                        # ctx + rowsum via ones-column of V
                        cps = ps.tile([DK + 1, QS], F32, tag="ctxp", name="cps")
                        for kt in range(KT):
                            nc.tensor.matmul(
                                cps[:], v_sb[kt][:, h, :], es[kt][:],
                                start=(kt == 0), stop=(kt == KT - 1))
                        rcp = psm.tile([65, QS], F32R, tag="rcp", bufs=1, name="rcp")
                        with nc.allow_low_precision(reason="f32r feeds PE"):
                            nc.vector.reciprocal(rcp[64:65, :], cps[DK:DK + 1, :])
                        bcp = ps.tile([128, QS], F32, tag="bcast", bufs=1, name="bcp")
                        nc.tensor.matmul(bcp[:], onesr64_sb[64:65, :],
                                         rcp[64:65, :], start=True, stop=True)
                        bc = psf.tile([128, QS], F32R, tag="bc", name="bc")
                        nc.scalar.copy(bc[:], bcp[:])
                        if po == 0:
                            nc.vector.tensor_mul(
                                ctx_sb[f][0:64, qb * QS:(qb + 1) * QS],
                                cps[0:DK, :], bc[0:DK, :])
                        else:
                            ctmp = psf.tile([64, QS], F32R, tag="ctmp", bufs=1,
                                            name="ctmp")
                            nc.vector.tensor_mul(ctmp[:], cps[0:DK, :],
                                                 bc[0:DK, :])
                            nc.sync.dma_start(
                                out=ctx_sb[f][64:128, qb * QS:(qb + 1) * QS],
                                in_=ctmp[:])
                        for kt in range(KT):
                            nc.vector.tensor_mul(es[kt][:], es[kt][:], bc[:])
                            nc.sync.dma_start(
                                out=attns[l, h, kt * 128:(kt + 1) * 128,
                                          qb * QS:(qb + 1) * QS],
                                in_=es[kt][:].bitcast(F32))

                    # ---- WO partial + AllReduce for this q-block ----
                    for dm in range(DM):
                        wot = pw.tile([128, HC, 128], F32R, tag="wo", name="wot")
                        nc.sync.dma_start(
                            out=wot,
                            in_=wo[l][:, dm * 128:(dm + 1) * 128].rearrange(
                                "(c p) n -> p c n", p=128))
                        acc = ps.tile([128, QS], F32, tag="mm", name="acc")
                        for hc in range(HC):
                            nc.tensor.matmul(
                                acc[:], wot[:, hc, :],
                                ctx_sb[hc][:, qb * QS:(qb + 1) * QS],
                                start=(hc == 0), stop=(hc == HC - 1))
                        st = pw.tile([128, QS], F32, tag="stg", name="st")
                        nc.vector.tensor_copy(st[:], acc[:])
                        nc.sync.dma_start(
                            out=ar_in[qb][dm * 128:(dm + 1) * 128, :],
                            in_=st[:])
                    nc.gpsimd.collective_compute(
                        "AllReduce", mybir.AluOpType.add, replica_groups=RG,
                        ins=[ar_in[qb][:].opt()], outs=[ar_out[qb][:].opt()])
                for qb in range(QB):
                    _ln_qb(nc, pa, pw, psm, ps, x_sb, ar_out[qb], ones_sb,
                           onesr_sb, eps_sb, nq, qb, QS, DC)

                # ---------------- FFN ----------------
                h_tags = ([f"z{i}" for i in range(DC)]
                          + [f"qt{i}" for i in range(FT)]
                          + [f"kt{i}" for i in range(FT)])
                h_sb = [pa.tile([128, nq], F32R, tag=h_tags[i], name=f"h{i}")
                        for i in range(F1)]
                ar_in2 = [pd.tile([D, QS], F32, tag=f"arin{qb}", name=f"ari2{qb}")
                          for qb in range(QB)]
                ar_out2 = [pd.tile([D, QS], F32, tag=f"arout{qb}", name=f"aro2{qb}")
                           for qb in range(QB)]
                for qb in range(QB):
                    for ft in range(F1):
                        w1t = pw.tile([128, DC, 128], F32R, tag="w1", name="w1t")
                        nc.sync.dma_start(
                            out=w1t,
                            in_=w1[l][:, ft * 128:(ft + 1) * 128].rearrange(
                                "(c p) n -> p c n", p=128))
                        acc = ps.tile([128, QS], F32, tag="mm", name="acc")
                        for dc in range(DC):
                            nc.tensor.matmul(
                                acc[:], w1t[:, dc, :],
                                x_sb[dc][:, qb * QS:(qb + 1) * QS],
                                start=(dc == 0), stop=(dc == DC - 1))
                        nc.vector.tensor_scalar_max(
                            h_sb[ft][:, qb * QS:(qb + 1) * QS], acc[:], 0.0)
                    for dm in range(DM):
                        w2t = pw.tile([128, F1, 128], F32R, tag="w2", name="w2t")
                        nc.sync.dma_start(
                            out=w2t,
                            in_=w2[l][:, dm * 128:(dm + 1) * 128].rearrange(
                                "(c p) n -> p c n", p=128))
                        acc = ps.tile([128, QS], F32, tag="mm", name="acc")
                        for fc in range(F1):
                            nc.tensor.matmul(
                                acc[:], w2t[:, fc, :],
                                h_sb[fc][:, qb * QS:(qb + 1) * QS],
                                start=(fc == 0), stop=(fc == F1 - 1))
                        st = pw.tile([128, QS], F32, tag="stg", name="st")
                        nc.vector.tensor_copy(st[:], acc[:])
                        nc.sync.dma_start(
                            out=ar_in2[qb][dm * 128:(dm + 1) * 128, :],
                            in_=st[:])
                    nc.gpsimd.collective_compute(
                        "AllReduce", mybir.AluOpType.add, replica_groups=RG,
                        ins=[ar_in2[qb][:].opt()], outs=[ar_out2[qb][:].opt()])
                for qb in range(QB):
                    _ln_qb(nc, pa, pw, psm, ps, x_sb, ar_out2[qb], ones_sb,
                           onesr_sb, eps_sb, nq, qb, QS, DC)

            for i in range(DC):
                nc.sync.dma_start(out=xout[i * 128:(i + 1) * 128, :],
                                  in_=x_sb[i][:])

    nc.compile()
    return nc


def _ln_qb(nc, pa, pw, psm, ps, x_sb, ar_out, ones_sb, onesr_sb, eps_sb,
           nq, qb, QS, DC):
    """x[:, qb block] <- LayerNorm(ar_out + x), transposed [d, q] layout.

    ar_out is this q-block's [D, QS] AllReduce result. Stats are column
    sums via a ones lhsT on the PE; the affine apply uses a/b row vectors
    broadcast across partitions via a K=1 matmul.
    """
    qs = slice(qb * QS, (qb + 1) * QS)
    z_sb = [pa.tile([128, nq], F32R, tag=f"z{i}", name=f"z{i}") if qb == 0
            else None for i in range(DC)]
    if qb == 0:
        _ln_qb._z = z_sb
    z_sb = _ln_qb._z
    sps1 = ps.tile([1, QS], F32, tag="score", name="st1")
    sps2 = ps.tile([1, QS], F32, tag="ctxp", name="st2")
    for dc in range(DC):
        zin = pw.tile([128, QS], F32, tag="stg", name="zin")
        nc.sync.dma_start(out=zin, in_=ar_out[dc * 128:(dc + 1) * 128, :])
        nc.vector.tensor_add(z_sb[dc][:, qs], zin[:], x_sb[dc][:, qs])
        z2 = pw.tile([128, QS], F32R, tag="z2", bufs=2, name="z2")
        nc.vector.tensor_mul(z2[:], z_sb[dc][:, qs], z_sb[dc][:, qs])
        nc.tensor.matmul(sps1[:], ones_sb[:], z_sb[dc][:, qs],
                         start=(dc == 0), stop=(dc == DC - 1))
        nc.tensor.matmul(sps2[:], ones_sb[:], z2[:],
                         start=(dc == 0), stop=(dc == DC - 1))
    mt = psm.tile([1, QS], F32, tag="lnm", bufs=1, name="mt")
    mst = psm.tile([1, QS], F32, tag="lnms", bufs=1, name="mst")
    vt = psm.tile([1, QS], F32, tag="lnvar", bufs=1, name="vt")
    a_t = psm.tile([1, QS], F32R, tag="av", bufs=1, name="a_t")
    b_t = psm.tile([1, QS], F32R, tag="bv", bufs=1, name="b_t")
    m, var, ms, a, b = mt[:], vt[:], mst[:], a_t[:], b_t[:]
    nc.vector.tensor_scalar_mul(m, sps1[:], 1.0 / D)
    nc.vector.tensor_scalar_mul(ms, sps2[:], 1.0 / D)
    nc.vector.tensor_mul(var, m, m)
    nc.vector.tensor_sub(var, ms, var)
    nc.scalar.activation(var, var, mybir.ActivationFunctionType.Sqrt,
                         bias=eps_sb[:], scale=1.0)
    with nc.allow_low_precision(reason="f32r feeds PE"):
        nc.vector.reciprocal(a, var)
    nc.vector.tensor_mul(b, m.bitcast(F32R), a)
    nc.vector.tensor_scalar_mul(b, b, -1.0)
    abc = ps.tile([128, QS], F32, tag="mm", name="abc")
    nc.tensor.matmul(abc[:], onesr_sb[:], a, start=True, stop=True)
    bbc = ps.tile([128, QS], F32, tag="mm", name="bbc")
    nc.tensor.matmul(bbc[:], onesr_sb[:], b, start=True, stop=True)
    for dc in range(DC):
        xs = x_sb[dc][:, qs]
        nc.vector.tensor_mul(xs, z_sb[dc][:, qs], abc[:])
        nc.vector.tensor_add(xs, xs, bbc[:])


_CACHE = {}


def _get_nc():
    if "nc" not in _CACHE:
        _CACHE["nc"] = build()
    return _CACHE["nc"]


def _prep_inputs(enc_inputs, emb, pe, WQ, WK, WV, WO, W1, W2):
    enc_inputs = np.asarray(enc_inputs)
    emb = np.asarray(emb, dtype=np.float32)
    pe = np.asarray(pe, dtype=np.float32)
    WQ = np.asarray(WQ, dtype=np.float32)
    WK = np.asarray(WK, dtype=np.float32)
    WV = np.asarray(WV, dtype=np.float32)
    WO = np.asarray(WO, dtype=np.float32)
    W1 = np.asarray(W1, dtype=np.float32)
    W2 = np.asarray(W2, dtype=np.float32)

    x0 = emb[enc_inputs] + pe[None, :L, :]          # [b, l, d] fp32
    scale = 1.0 / math.sqrt(DK)

    in_maps = []
    for c in range(N_CORES):
        b, half = c // 2, c % 2
        hs = slice(half * FH, (half + 1) * FH)
        fs = slice(half * DFF_PER, (half + 1) * DFF_PER)
        pad = enc_inputs[b] == 0
        im = {
            "x0": _round_tf(x0[b].T),
            "cones": np.ones((128, 128), dtype=np.float32),
            "mask": np.ascontiguousarray(pad.reshape(L // 128, 128).T
                                         .astype(np.float32) * NEG),
        }
        for l in range(NL):
            im[f"wq{l}"] = _round_tf(WQ[l][:, hs] * scale)
            im[f"wk{l}"] = _round_tf(WK[l][:, hs])
            im[f"wv{l}"] = _round_tf(WV[l][:, hs])
            im[f"wo{l}"] = _round_tf(WO[l][hs, :])
            im[f"w1{l}"] = _round_tf(W1[l][:, fs])
            im[f"w2{l}"] = _round_tf(W2[l][fs, :])
        in_maps.append(im)
    return in_maps


def _run(in_maps, trace=False, **kw):
    nc = _get_nc()
    return bass_utils.run_bass_kernel_spmd(nc, in_maps, list(range(N_CORES)),
                                           trace=trace, **kw)


def kernel(enc_inputs, emb, pe, WQ, WK, WV, WO, W1, W2):
    in_maps = _prep_inputs(enc_inputs, emb, pe, WQ, WK, WV, WO, W1, W2)
    res = _run(in_maps)

    x = np.empty((B, L, D), dtype=np.float32)
    attns = np.empty((NL, B, HEADS, L, L), dtype=np.float32)
    for c in range(N_CORES):
        b, half = c // 2, c % 2
        r = res.results[c]
        if half == 0:
            x[b] = r["xout"].T
        # attns device layout: [layer, local_head, k, q] -> [l, b, h, q, k]
        a = r["attns"]
        attns[:, b, half * H_PER:(half + 1) * H_PER] = np.swapaxes(a, 2, 3)
    return x, attns


if __name__ == "__main__":
    print("building...")
    nc = _get_nc()
    print("built ok")


# revision 35
# speedup vs baseline: 1.0596x; 1.0596x over previous
"""Trainium2 Bass kernel for a 2-layer Transformer encoder (nn_Encoder).

Sharding: data-parallel over batch (4 pairs of cores) x tensor-parallel
(2-way) within each pair: heads 0-7 / 8-15 for attention, d_ff columns
0-2047 / 2048-4095 for the FFN, with a pairwise AllReduce after the WO
and W2 projections.

Device layout: activations are kept transposed (X_T: [d_model, tokens])
so every matmul contracts over the partition dim. Attention scores are
computed as S_T [keys, queries]; softmax rowsums come for free from a
ones-column appended to V; probabilities are normalized on-device and
written to DRAM in [k, q] layout (host transposes while assembling).

All matmuls run in float32r (fp32 rounded to 11-bit mantissa, full PE
rate); end-to-end max rel err vs the fp32 reference is ~6e-4.
"""
import sys

sys.path.insert(0, "/opt/trn_rl_repo")

import math
import numpy as np

import concourse.bass as bass
import concourse.tile as tile
from concourse import bacc, mybir
from concourse import bass_utils

F32 = mybir.dt.float32
F32R = mybir.dt.float32r

N_CORES = 8
B, L, D, DK, HEADS, DFF, NL = 4, 1024, 1024, 64, 16, 4096, 2
H_PER = HEADS // 2          # heads per core
FH = H_PER * DK             # 512 head-feature rows per core
DFF_PER = DFF // 2          # 2048 ffn cols per core
EPS = 1e-5
NEG = -1e30


def _round_tf(x):
    """Round fp32 -> float32r (RNE to 11-bit mantissa). Matches walrus."""
    x = np.ascontiguousarray(x, dtype=np.float32)
    u = x.view(np.uint32)
    r = (u + np.uint32(0x7FF) + ((u >> np.uint32(12)) & np.uint32(1))) & np.uint32(
        0xFFFFF000
    )
    return r.view(np.float32)


def build(l_seq=L, n_layers=NL, qs=None):
    """Build + compile the per-core SPMD program. Same program on all cores."""
    nq = l_seq                      # queries per core (all of them, TP over heads)
    QS = qs if qs is not None else min(512, nq)  # q-block width
    QB = nq // QS                   # number of query blocks
    KT = l_seq // 128               # 128-row key tiles
    TT = l_seq // 128               # token tiles for V
    DC = D // 128                   # d_model chunks
    FT = FH // 128                  # f-tiles for Q_T/K_T (4)
    HC = FH // 128                  # hdv chunks for WO (4)
    F1 = DFF_PER // 128             # dff tiles (16)
    DM = D // 128                   # output d_model tiles (8)

    nc = bacc.Bacc("TRN2", target_bir_lowering=False, debug=False,
                   num_devices=N_CORES)

    x0 = nc.dram_tensor("x0", [D, l_seq], F32R, kind="ExternalInput").ap()
    mask = nc.dram_tensor("mask", [128, KT], F32, kind="ExternalInput").ap()
    cones = nc.dram_tensor("cones", [128, 128], F32R, kind="ExternalInput").ap()
    wq, wk, wv, wo, w1, w2 = [], [], [], [], [], []
    for l in range(n_layers):
        wq.append(nc.dram_tensor(f"wq{l}", [D, FH], F32R, kind="ExternalInput").ap())
        wk.append(nc.dram_tensor(f"wk{l}", [D, FH], F32R, kind="ExternalInput").ap())
        wv.append(nc.dram_tensor(f"wv{l}", [D, FH], F32R, kind="ExternalInput").ap())
        wo.append(nc.dram_tensor(f"wo{l}", [FH, D], F32R, kind="ExternalInput").ap())
        w1.append(nc.dram_tensor(f"w1{l}", [D, DFF_PER], F32R, kind="ExternalInput").ap())
        w2.append(nc.dram_tensor(f"w2{l}", [DFF_PER, D], F32R, kind="ExternalInput").ap())
    attns = nc.dram_tensor("attns", [n_layers, H_PER, l_seq, nq], F32,
                           kind="ExternalOutput").ap()
    xout = nc.dram_tensor("xout", [D, nq], F32R, kind="ExternalOutput").ap()

    RG = [[0, 1], [2, 3], [4, 5], [6, 7]]

    with tile.TileContext(nc) as tc:
        with (
            tc.tile_pool(name="persist", bufs=1) as pp,
            tc.tile_pool(name="acts", bufs=1) as pa,
            tc.tile_pool(name="wload", bufs=2) as pw,
            tc.tile_pool(name="soft", bufs=1) as psf,
            tc.tile_pool(name="small", bufs=1) as psm,
            tc.tile_pool(name="psum", bufs=2, space="PSUM") as ps,
            tc.tile_pool(name="dram", bufs=4, space="DRAM") as pd,
        ):
            # persistent tiles
            x_sb = [pp.tile([128, l_seq], F32R, tag=f"x{i}", name=f"x{i}")
                    for i in range(DC)]
            mask_sb = pp.tile([128, KT], F32, tag="mask")
            ones_sb = pp.tile([128, 1], F32R, tag="ones")     # LN stats lhsT
            onesr_sb = pp.tile([1, 128], F32R, tag="onesr")   # broadcast lhsT
            onesr64_sb = pp.tile([65, 128], F32R, tag="onesr64")  # row 64 ones
            eps_sb = pp.tile([1, 1], F32, tag="eps")

            nc.sync.dma_start(out=ones_sb, in_=cones[:, 0:1])
            nc.sync.dma_start(out=onesr_sb, in_=cones[0:1, :])
            nc.sync.dma_start(out=onesr64_sb[64:65, :], in_=cones[0:1, :])
            nc.vector.memset(eps_sb, EPS)
            nc.sync.dma_start(out=mask_sb, in_=mask)
            for i in range(DC):
                nc.sync.dma_start(out=x_sb[i], in_=x0[i * 128:(i + 1) * 128, :])

            for l in range(n_layers):
                # ---------------- QKV projections ----------------
                qt_sb = [pa.tile([128, nq], F32R, tag=f"qt{i}", name=f"qt{i}")
                         for i in range(FT)]
                kt_sb = [pa.tile([128, nq], F32R, tag=f"kt{i}", name=f"kt{i}")
                         for i in range(FT)]
                v_sb = [pa.tile([128, H_PER, DK + 1], F32R, tag=f"v{i}",
                                name=f"v{i}") for i in range(TT)]

                for (w_ap, dst) in ((wq[l], qt_sb), (wk[l], kt_sb)):
                    for ft in range(FT):
                        wt = pw.tile([128, DC, 128], F32R, tag="wqk", name="wt")
                        nc.sync.dma_start(
                            out=wt,
                            in_=w_ap[:, ft * 128:(ft + 1) * 128].rearrange(
                                "(c p) n -> p c n", p=128))
                        for qb in range(QB):
                            acc = ps.tile([128, QS], F32, tag="mm", name="acc")
                            for dc in range(DC):
                                nc.tensor.matmul(
                                    acc[:], wt[:, dc, :],
                                    x_sb[dc][:, qb * QS:(qb + 1) * QS],
                                    start=(dc == 0), stop=(dc == DC - 1))
                            nc.vector.tensor_copy(
                                dst[ft][:, qb * QS:(qb + 1) * QS], acc[:])

                # V natural [tok, h*(dk+1)] with ones column per head
                wv_hold = [pa.tile([128, FH], F32R, tag=f"z{dc}", name=f"wvh{dc}")
                           for dc in range(DC)]
                for dc in range(DC):
                    nc.sync.dma_start(out=wv_hold[dc],
                                      in_=wv[l][dc * 128:(dc + 1) * 128, :])
                for tt in range(TT):
                    acc = ps.tile([128, FH], F32, tag="mm", name="acc")
                    for dc in range(DC):
                        nc.tensor.matmul(
                            acc[:], x_sb[dc][:, tt * 128:(tt + 1) * 128],
                            wv_hold[dc][:], start=(dc == 0), stop=(dc == DC - 1))
                    nc.sync.dma_start(out=v_sb[tt][:, :, DK:DK + 1],
                                      in_=cones[:, 0:H_PER].rearrange(
                                          "p (h o) -> p h o", o=1))
                    nc.vector.tensor_copy(
                        v_sb[tt][:, :, 0:DK],
                        acc[:].rearrange("p (h d) -> p h d", h=H_PER))

                # ---------------- attention ----------------
                ctx_sb = [pa.tile([128, nq], F32R, tag=f"ctx{i}", name=f"ctx{i}")
                          for i in range(HC)]
                ar_in = [pd.tile([D, QS], F32, tag=f"arin{qb}", name=f"ari{qb}")
                         for qb in range(QB)]
                ar_out = [pd.tile([D, QS], F32, tag=f"arout{qb}", name=f"aro{qb}")
                          for qb in range(QB)]

                for qb in range(QB):
                    for h in range(H_PER):
                        f, po = h // 2, (h % 2) * 64
                        es = [psf.tile([128, QS], F32R, tag="es", bufs=KT + 2,
                                       name=f"es{kt}") for kt in range(KT)]
                        for kt in range(KT):
                            sps = ps.tile([128, QS], F32, tag="score", name="sps")
                            nc.tensor.matmul(
                                sps[:],
                                kt_sb[f][po:po + 64, kt * 128:(kt + 1) * 128],
                                qt_sb[f][po:po + 64, qb * QS:(qb + 1) * QS],
                                start=True, stop=True)
                            nc.scalar.activation(
                                es[kt][:], sps[:],
                                mybir.ActivationFunctionType.Exp,
                                bias=mask_sb[:, kt:kt + 1], scale=1.0)
                        # ctx + rowsum via ones-column of V
                        cps = ps.tile([DK + 1, QS], F32, tag="ctxp", name="cps")
                        for kt in range(KT):
                            nc.tensor.matmul(
                                cps[:], v_sb[kt][:, h, :], es[kt][:],
                                start=(kt == 0), stop=(kt == KT - 1))
                        rcp = psm.tile([65, QS], F32R, tag="rcp", bufs=1, name="rcp")
                        with nc.allow_low_precision(reason="f32r feeds PE"):
                            nc.vector.reciprocal(rcp[64:65, :], cps[DK:DK + 1, :])
                        bcp = ps.tile([128, QS], F32, tag="bcast", bufs=1, name="bcp")
                        nc.tensor.matmul(bcp[:], onesr64_sb[64:65, :],
                                         rcp[64:65, :], start=True, stop=True)
                        bc = psf.tile([128, QS], F32R, tag="bc", name="bc")
                        nc.scalar.copy(bc[:], bcp[:])
                        if po == 0:
                            nc.vector.tensor_mul(
                                ctx_sb[f][0:64, qb * QS:(qb + 1) * QS],
                                cps[0:DK, :], bc[0:DK, :])
                        else:
                            ctmp = psf.tile([64, QS], F32R, tag="ctmp", bufs=1,
                                            name="ctmp")
                            nc.vector.tensor_mul(ctmp[:], cps[0:DK, :],
                                                 bc[0:DK, :])
                            nc.sync.dma_start(
                                out=ctx_sb[f][64:128, qb * QS:(qb + 1) * QS],
                                in_=ctmp[:])
                        for kt in range(KT):
                            nc.vector.tensor_mul(es[kt][:], es[kt][:], bc[:])
                            nc.sync.dma_start(
                                out=attns[l, h, kt * 128:(kt + 1) * 128,
                                          qb * QS:(qb + 1) * QS],
                                in_=es[kt][:].bitcast(F32))

                    # ---- WO partial + AllReduce for this q-block ----
                    for dm in range(DM):
                        wot = pw.tile([128, HC, 128], F32R, tag="wo", bufs=3, name="wot")
                        nc.sync.dma_start(
                            out=wot,
                            in_=wo[l][:, dm * 128:(dm + 1) * 128].rearrange(
                                "(c p) n -> p c n", p=128))
                        acc = ps.tile([128, QS], F32, tag="mm", name="acc")
                        for hc in range(HC):
                            nc.tensor.matmul(
                                acc[:], wot[:, hc, :],
                                ctx_sb[hc][:, qb * QS:(qb + 1) * QS],
                                start=(hc == 0), stop=(hc == HC - 1))
                        st = pw.tile([128, QS], F32, tag="stg", name="st")
                        nc.vector.tensor_copy(st[:], acc[:])
                        nc.sync.dma_start(
                            out=ar_in[qb][dm * 128:(dm + 1) * 128, :],
                            in_=st[:])
                    nc.gpsimd.collective_compute(
                        "AllReduce", mybir.AluOpType.add, replica_groups=RG,
                        ins=[ar_in[qb][:].opt()], outs=[ar_out[qb][:].opt()])
                for qb in range(QB):
                    _ln_qb(nc, pa, pw, psm, ps, x_sb, ar_out[qb], ones_sb,
                           onesr_sb, eps_sb, nq, qb, QS, DC)

                # ---------------- FFN ----------------
                h_tags = ([f"z{i}" for i in range(DC)]
                          + [f"qt{i}" for i in range(FT)]
                          + [f"kt{i}" for i in range(FT)])
                h_sb = [pa.tile([128, nq], F32R, tag=h_tags[i], name=f"h{i}")
                        for i in range(F1)]
                ar_in2 = [pd.tile([D, QS], F32, tag=f"arin{qb}", name=f"ari2{qb}")
                          for qb in range(QB)]
                ar_out2 = [pd.tile([D, QS], F32, tag=f"arout{qb}", name=f"aro2{qb}")
                           for qb in range(QB)]
                for qb in range(QB):
                    for ft in range(F1):
                        w1t = pw.tile([128, DC, 128], F32R, tag="w1", name="w1t")
                        nc.sync.dma_start(
                            out=w1t,
                            in_=w1[l][:, ft * 128:(ft + 1) * 128].rearrange(
                                "(c p) n -> p c n", p=128))
                        acc = ps.tile([128, QS], F32, tag="mm", name="acc")
                        for dc in range(DC):
                            nc.tensor.matmul(
                                acc[:], w1t[:, dc, :],
                                x_sb[dc][:, qb * QS:(qb + 1) * QS],
                                start=(dc == 0), stop=(dc == DC - 1))
                        nc.vector.tensor_scalar_max(
                            h_sb[ft][:, qb * QS:(qb + 1) * QS], acc[:], 0.0)
                    for dm in range(DM):
                        w2t = pw.tile([128, F1, 128], F32R, tag="w2", name="w2t")
                        nc.sync.dma_start(
                            out=w2t,
                            in_=w2[l][:, dm * 128:(dm + 1) * 128].rearrange(
                                "(c p) n -> p c n", p=128))
                        acc = ps.tile([128, QS], F32, tag="mm", name="acc")
                        for fc in range(F1):
                            nc.tensor.matmul(
                                acc[:], w2t[:, fc, :],
                                h_sb[fc][:, qb * QS:(qb + 1) * QS],
                                start=(fc == 0), stop=(fc == F1 - 1))
                        st = pw.tile([128, QS], F32, tag="stg", name="st")
                        nc.vector.tensor_copy(st[:], acc[:])
                        nc.sync.dma_start(
                            out=ar_in2[qb][dm * 128:(dm + 1) * 128, :],
                            in_=st[:])
                    nc.gpsimd.collective_compute(
                        "AllReduce", mybir.AluOpType.add, replica_groups=RG,
                        ins=[ar_in2[qb][:].opt()], outs=[ar_out2[qb][:].opt()])
                for qb in range(QB):
                    _ln_qb(nc, pa, pw, psm, ps, x_sb, ar_out2[qb], ones_sb,
                           onesr_sb, eps_sb, nq, qb, QS, DC)

            for i in range(DC):
                nc.sync.dma_start(out=xout[i * 128:(i + 1) * 128, :],
                                  in_=x_sb[i][:])

    nc.compile()
    return nc


def _ln_qb(nc, pa, pw, psm, ps, x_sb, ar_out, ones_sb, onesr_sb, eps_sb,
           nq, qb, QS, DC):
    """x[:, qb block] <- LayerNorm(ar_out + x), transposed [d, q] layout.

    ar_out is this q-block's [D, QS] AllReduce result. Stats are column
    sums via a ones lhsT on the PE; the affine apply uses a/b row vectors
    broadcast across partitions via a K=1 matmul.
    """
    qs = slice(qb * QS, (qb + 1) * QS)
    z_sb = [pa.tile([128, nq], F32R, tag=f"z{i}", name=f"z{i}") if qb == 0
            else None for i in range(DC)]
    if qb == 0:
        _ln_qb._z = z_sb
    z_sb = _ln_qb._z
    sps1 = ps.tile([1, QS], F32, tag="score", name="st1")
    sps2 = ps.tile([1, QS], F32, tag="ctxp", name="st2")
    for dc in range(DC):
        zin = pw.tile([128, QS], F32, tag="stg", name="zin")
        nc.sync.dma_start(out=zin, in_=ar_out[dc * 128:(dc + 1) * 128, :])
        nc.vector.tensor_add(z_sb[dc][:, qs], zin[:], x_sb[dc][:, qs])
        z2 = pw.tile([128, QS], F32R, tag="z2", bufs=1, name="z2")
        nc.vector.tensor_mul(z2[:], z_sb[dc][:, qs], z_sb[dc][:, qs])
        nc.tensor.matmul(sps1[:], ones_sb[:], z_sb[dc][:, qs],
                         start=(dc == 0), stop=(dc == DC - 1))
        nc.tensor.matmul(sps2[:], ones_sb[:], z2[:],
                         start=(dc == 0), stop=(dc == DC - 1))
    mt = psm.tile([1, QS], F32, tag="lnm", bufs=1, name="mt")
    mst = psm.tile([1, QS], F32, tag="lnms", bufs=1, name="mst")
    a_t = psm.tile([1, QS], F32R, tag="av", bufs=1, name="a_t")
    b_t = psm.tile([1, QS], F32R, tag="bv", bufs=1, name="b_t")
    m, ms, a, b = mt[:], mst[:], a_t[:], b_t[:]
    nc.vector.tensor_scalar_mul(m, sps1[:], 1.0 / D)
    nc.vector.tensor_scalar_mul(ms, sps2[:], 1.0 / D)
    nc.vector.tensor_mul(sps1[:], m, m)
    nc.vector.tensor_sub(ms, ms, sps1[:])
    nc.scalar.activation(ms, ms, mybir.ActivationFunctionType.Sqrt,
                         bias=eps_sb[:], scale=1.0)
    with nc.allow_low_precision(reason="f32r feeds PE"):
        nc.vector.reciprocal(a, ms)
    nc.vector.tensor_mul(b, m.bitcast(F32R), a)
    nc.vector.tensor_scalar_mul(b, b, -1.0)
    abc = ps.tile([128, QS], F32, tag="mm", name="abc")
    nc.tensor.matmul(abc[:], onesr_sb[:], a, start=True, stop=True)
    bbc = ps.tile([128, QS], F32, tag="mm", name="bbc")
    nc.tensor.matmul(bbc[:], onesr_sb[:], b, start=True, stop=True)
    for dc in range(DC):
        xs = x_sb[dc][:, qs]
        nc.vector.tensor_mul(xs, z_sb[dc][:, qs], abc[:])
        nc.vector.tensor_add(xs, xs, bbc[:])


_CACHE = {}


def _get_nc():
    if "nc" not in _CACHE:
        _CACHE["nc"] = build()
    return _CACHE["nc"]


def _prep_inputs(enc_inputs, emb, pe, WQ, WK, WV, WO, W1, W2):
    enc_inputs = np.asarray(enc_inputs)
    emb = np.asarray(emb, dtype=np.float32)
    pe = np.asarray(pe, dtype=np.float32)
    WQ = np.asarray(WQ, dtype=np.float32)
    WK = np.asarray(WK, dtype=np.float32)
    WV = np.asarray(WV, dtype=np.float32)
    WO = np.asarray(WO, dtype=np.float32)
    W1 = np.asarray(W1, dtype=np.float32)
    W2 = np.asarray(W2, dtype=np.float32)

    x0 = emb[enc_inputs] + pe[None, :L, :]          # [b, l, d] fp32
    scale = 1.0 / math.sqrt(DK)

    in_maps = []
    for c in range(N_CORES):
        b, half = c // 2, c % 2
        hs = slice(half * FH, (half + 1) * FH)
        fs = slice(half * DFF_PER, (half + 1) * DFF_PER)
        pad = enc_inputs[b] == 0
        im = {
            "x0": _round_tf(x0[b].T),
            "cones": np.ones((128, 128), dtype=np.float32),
            "mask": np.ascontiguousarray(pad.reshape(L // 128, 128).T
                                         .astype(np.float32) * NEG),
        }
        for l in range(NL):
            im[f"wq{l}"] = _round_tf(WQ[l][:, hs] * scale)
            im[f"wk{l}"] = _round_tf(WK[l][:, hs])
            im[f"wv{l}"] = _round_tf(WV[l][:, hs])
            im[f"wo{l}"] = _round_tf(WO[l][hs, :])
            im[f"w1{l}"] = _round_tf(W1[l][:, fs])
            im[f"w2{l}"] = _round_tf(W2[l][fs, :])
        in_maps.append(im)
    return in_maps


def _run(in_maps, trace=False, **kw):
    nc = _get_nc()
    return bass_utils.run_bass_kernel_spmd(nc, in_maps, list(range(N_CORES)),
                                           trace=trace, **kw)


def kernel(enc_inputs, emb, pe, WQ, WK, WV, WO, W1, W2):
    in_maps = _prep_inputs(enc_inputs, emb, pe, WQ, WK, WV, WO, W1, W2)
    res = _run(in_maps)

    x = np.empty((B, L, D), dtype=np.float32)
    attns = np.empty((NL, B, HEADS, L, L), dtype=np.float32)
    for c in range(N_CORES):
        b, half = c // 2, c % 2
        r = res.results[c]
        if half == 0:
            x[b] = r["xout"].T
        # attns device layout: [layer, local_head, k, q] -> [l, b, h, q, k]
        a = r["attns"]
        attns[:, b, half * H_PER:(half + 1) * H_PER] = np.swapaxes(a, 2, 3)
    return x, attns


if __name__ == "__main__":
    print("building...")
    nc = _get_nc()
    print("built ok")
